# revision 30
# baseline (speedup 1.0000x reference)
"""Adaptive-input-embedding Bass kernel for one TRN2 chip (8 NeuronCores). v2

Token-parallel across the 8 cores: tokens are grouped by bucket, sorted by
table index, and dealt as contiguous runs, so every core processes ~4096
tokens with identical compile-time structure.

Buckets 0/1 (300+2700 rows, ~1.1% of tokens) are folded host-side: the
host precomputes P_i = emb_i @ W_i once and writes those token rows during
unshard, so the device only handles buckets 2/3 (~99% of tokens).

Device path: each core's contiguous table windows (bucket 2: ~3.6k rows
of 256, bucket 3: ~30k rows of 128, both bf16) are SBUF-RESIDENT constants
loaded once outside the timing loop. Per body, SWDGE transpose-gathers run
SBUF->SBUF directly into lhsT layout (row r lives at partition r%128, rank
r//128 of the resident window). The gather stream's binding resource is Q7
descriptor generation (~8.7ns/row measured; the cost model's 0.34ns/desc
is wrong for gather-type gen) — spreading the calls across all 4 SWDGE
queue contexts (num_swdge_queues=4) parallelizes gen 2.9x, after which
gathers (~15us) hide under the PE stream (~20us). Matmuls against resident
bf16 W chunks accumulate into PSUM f32; PSUM copies to SBUF bf16 alternate
DVE/ACT 4:7 (PSUM has a single engine read port, so copies are
PSUM-read-bound at ~1.1/1.5us per 128x1024 tile; 4 single-bank... 4
double-bank PSUM bufs keep the mm->copy->WAR cycle off the critical path).
Output rows are stored in 4-tile batches alternating the two HWDGE rings
at the HBM roofline (~357GB/s measured): tokens hitting duplicate table
rows share one slot (host scatter fans the row out), so slots ~= unique
rows (~3968 vs 4096 tokens); the bucket-2 tail and bucket-3 head share one
mixed 128-slot tile whose PSUM accumulates both matmuls, with explicit
zero rows appended to each window keeping the off-bucket lhsT columns
zero. Residual ~8us above the
store roofline is SDMA-level interference between the gather's xbar
transpose transfers and the store stream (invariant to packet splitting,
chunk size, and queue layout; an HBM-staging + HWDGE-xbar-transpose
variant (hb=1) was slower). The host scatters returned rows to token
positions (unshard).
"""

import sys

import numpy as np

try:
    import concourse  # noqa: F401
except ImportError:
    sys.path.insert(0, "/opt/trn_rl_repo")

import ml_dtypes
from concourse import bacc, mybir, tile
from concourse.bass_utils import run_bass_kernel_spmd

BUCKETS = (0, 300, 3000, 30000, 267734)
SIZES = [BUCKETS[i + 1] - BUCKETS[i] for i in range(4)]
D = 1024
DS = [1024, 512, 256, 128]  # embedding dim per bucket
NCORES = 8
P = 128
SEQ = 4096
NTOK = NCORES * SEQ
SUB = 32768  # rows addressable by one int16 gather call
GCAP = 768  # >=1024 idxs in one SWDGE gather wedges the device

MODE = "v2"

_BF16 = ml_dtypes.bfloat16

_cache: dict = {}


def _r16(v):
    return -(-int(v) // 16) * 16


def _r128(v):
    return -(-int(v) // 128) * 128


class Plan:
    pass


def _plan(x):
    """Bucketing + even dealing of buckets 2/3 across the cores.

    Tokens of each bucket are sorted by table index and dealt as contiguous
    runs, so each core's gather indices span a narrow window of the table
    (int16-addressable, SBUF-resident). Buckets 0/1 go to the host path."""
    xf = x.reshape(-1).astype(np.int64)
    assert xf.shape[0] == NTOK
    b_arr = np.asarray(BUCKETS)
    bkt = np.clip(np.searchsorted(b_arr, xf, side="right") - 1, 0, 3)
    loc = xf - b_arr[bkt]

    p = Plan()
    # host path: buckets 0/1 (tiny token counts; host writes rows directly)
    p.hpos, p.hloc = [], []
    for b in (0, 1):
        pos = np.nonzero(bkt == b)[0]
        p.hpos.append(pos)
        p.hloc.append(loc[pos])

    percore = {}
    wbase = np.zeros((4, NCORES), np.int64)
    alloc = {}
    span = {}
    for b in (2, 3):
        pos = np.nonzero(bkt == b)[0]
        pos = pos[np.argsort(loc[pos], kind="stable")]
        n = pos.size
        cnt = np.full(NCORES, n // NCORES)
        cnt[: n % NCORES] += 1
        cuts = np.concatenate([[0], np.cumsum(cnt)])

        def spans(cuts_):
            sp, mx = 0, 0
            for c in range(NCORES):
                pc = pos[cuts_[c] : cuts_[c + 1]]
                if pc.size:
                    sp = max(sp, int(loc[pc[-1]] - loc[pc[0]]) + 1)
                    mx = max(mx, pc.size)
            return sp, mx

        sp, mx = spans(cuts)
        if b == 3 and sp > SUB - 256:
            # skewed distribution: balanced cuts straddle too-wide ranges;
            # fall back to fixed-boundary cuts (unbalanced counts but
            # windows stay int16-addressable)
            edges = np.searchsorted(loc[pos], np.arange(1, NCORES) * (SUB - 256))
            cuts = np.concatenate([[0], edges, [n]])
            sp, mx = spans(cuts)
        mxu = 16
        for c in range(NCORES):
            pc = pos[cuts[c] : cuts[c + 1]]
            lc = loc[pc]
            # dedup: tokens hitting the same table row share one slot (the
            # host scatter fans the row out to all their positions) — ~6%
            # fewer gathered rows AND stored slots
            if pc.size:
                ulc, inv = np.unique(lc, return_inverse=True)
                wbase[b, c] = ulc[0]
            else:
                ulc = np.zeros(0, np.int64)
                inv = np.zeros(0, np.int64)
            percore[(b, c)] = (pc, ulc, inv)
            mxu = max(mxu, ulc.size)
        alloc[b] = _r16(mxu)
        span[b] = max(sp, 1)

    # window shapes (global, compile-time): >=1 zero row, multiple of 128
    R2 = _r128(span[2] + 1)
    R3 = _r128(span[3] + 1)
    assert R2 <= 8192 and R3 <= SUB, (R2, R3)
    Z2, Z3 = R2 - 1, R3 - 1  # guaranteed-zero rows (windows zero-padded)

    # slot layout: b2 slots [0, A2), b3 slots [A2, ntot). The b3 gather
    # column space starts at the last 128-aligned boundary <= A2 so the
    # mixed tile accumulates b2's tail and b3's head (zero-row padding on
    # both sides keeps the off-bucket columns zero).
    A2 = alloc[2]
    T3 = (A2 // P) * P
    lead = A2 - T3
    G2 = T3 + (P if lead else 0)  # b2 gather count = r128(A2)
    N3 = _r128(lead + alloc[3])  # b3 gather count
    ntot = T3 + N3
    p.A2, p.T3, p.lead, p.G2, p.N3, p.ntot = A2, T3, lead, G2, N3, ntot
    p.R2, p.R3, p.Z2, p.Z3 = R2, R3, Z2, Z3
    p.t_total = ntot // P
    p.wbase = wbase

    NI = G2 + N3
    gidx = np.zeros((NCORES, P, NI // 16), np.int16)
    p.asm = []  # per-core (token positions, their slots) for the unshard
    for c in range(NCORES):
        idxs = np.empty(NI, np.int64)
        pc2, u2, inv2 = percore[(2, c)]
        n2 = u2.size
        idxs[:G2] = Z2
        idxs[:n2] = u2 - wbase[2, c]
        pc3, u3, inv3 = percore[(3, c)]
        n3 = u3.size
        idxs[G2:] = Z3
        idxs[G2 + lead : G2 + lead + n3] = u3 - wbase[3, c]
        p.asm.append(
            (np.concatenate([pc2, pc3]), np.concatenate([inv2, A2 + inv3]))
        )
        ii = np.arange(NI)
        cols = ii // 16
        rows = ii % 16
        for g in range(8):  # replicate across the 8 groups of 16 partitions
            gidx[c, g * 16 + rows, cols] = idxs.astype(np.int16)
    p.gidx = gidx
    return p


def _build(plan, mode=MODE, repeat=1, loop_n=None, gbatch=4, zbufs=6, psbufs=4,
           parts="gmcs", u=50, stag=False, gcap=GCAP, cr=(4, 7), spk=1, nq=4,
           se=2, ph=0, tr=0, cb=0, hb=0, ts=2):
    """Build + compile the SPMD Bass program.

    repeat>1 re-emits the whole body; loop_n wraps the body in a HW For_i
    loop (both used only for differential timing). parts selects body op
    groups (g=gathers, m=matmuls, c=psum copies, s=stores). cr=(a,b):
    a of every b psum copies go to DVE, the rest to ACT."""
    bf16 = mybir.dt.bfloat16
    f32 = mybir.dt.float32
    t_total = plan.t_total
    T3, G2, N3, lead = plan.T3, plan.G2, plan.N3, plan.lead
    NI = G2 + N3

    nc = bacc.Bacc(None, target_bir_lowering=False, num_swdge_queues=nq)
    if hb:
        # raw table windows stay in HBM; gathers are non-transpose (cheap
        # single-partition row writes), lhsT built by one HWDGE xbar
        # transpose per body (the efficient 261+ GB/s path)
        e3_d = nc.declare_dram_parameter("e3r", [plan.R3, DS[3]], bf16,
                                         isOutput=False)
        e2_d = nc.declare_dram_parameter("e2r", [plan.R2, DS[2]], bf16,
                                         isOutput=False)
    else:
        e3_d = nc.declare_dram_parameter("e3", [P, plan.R3], bf16, isOutput=False)
        e2_d = nc.declare_dram_parameter("e2", [P, plan.R2 * 2], bf16,
                                         isOutput=False)
    wcat_d = nc.declare_dram_parameter("wcat", [P, 3 * D], bf16, isOutput=False)
    gidx_d = nc.declare_dram_parameter("gidx", [P, NI // 16], mybir.dt.int16,
                                       isOutput=False)
    # partition-major: slot s lives at out[s % 128, s // 128, :] so each
    # partition's store stream is contiguous (few, large descriptors)
    out_d = nc.declare_dram_parameter("out", [P, t_total, D], bf16, isOutput=True)

    with tile.TileContext(nc) as tc:
        with (
            tc.tile_pool(name="const", bufs=1) as cp,
            tc.tile_pool(name="gbuf", bufs=2) as gp,
            tc.tile_pool(name="zbuf", bufs=zbufs) as zp,
            tc.tile_pool(name="ps", bufs=psbufs, space="PSUM") as pp,
        ):
            gidx = cp.tile([P, NI // 16], mybir.dt.int16)
            nc.gpsimd.dma_start(out=gidx[:], in_=gidx_d[:])
            wcat = cp.tile([P, 3 * D], bf16)
            nc.sync.dma_start(out=wcat[:], in_=wcat_d[:])
            if not hb:
                e3s = cp.tile([P, plan.R3], bf16)
                nc.sync.dma_start(out=e3s[:], in_=e3_d[:])
                e2s = cp.tile([P, plan.R2 * 2], bf16)
                nc.scalar.dma_start(out=e2s[:], in_=e2_d[:])

            def tile_chunks(t):
                # (src, chunk, col): lhsT = src[:, chunk, col:col+P]
                if t < T3 // P:
                    return [(2, 0, t * P), (2, 1, t * P)]
                if lead and t == T3 // P:
                    return [(2, 0, T3), (2, 1, T3), (3, 0, 0)]
                return [(3, 0, t * P - T3)]

            nb3 = N3 // P
            nb2 = G2 // P

            def body(_iv=None):
                # double-buffered gather tiles: body N+1's gathers overlap
                # body N's matmul/copy/store consume phase
                if hb:
                    st3 = gp.tile([P, nb3, DS[3]], bf16, tag="S3")
                    st2 = gp.tile([P, nb2, DS[2]], bf16, tag="S2")
                    gt3 = gp.tile([P, nb3, P], bf16, tag="G3")  # lhsT blocks
                    l2 = gp.tile([P, 2 * nb2, P], bf16, tag="L2")  # half-rows
                else:
                    gt3 = gp.tile([P, 1, N3], bf16, tag="G3")
                    l2 = gp.tile([P, 2, G2], bf16, tag="L2")
                if "g" in parts:
                    qn = [0]

                    def nextq():
                        q = qn[0] % nq
                        qn[0] += 1
                        return q

                    if hb:
                        # HBM non-transpose row gathers into token-major
                        # staging; one HWDGE xbar transpose per target builds
                        # the lhsT blocks (out[d, b, p] = st[p, b*128+d])
                        nc.gpsimd.dma_gather(
                            out_ap=st2[:, :, :],
                            in_ap=e2_d[:],
                            idxs_ap=gidx[:, 0 : G2 // 16],
                            num_idxs=G2,
                            num_idxs_reg=G2,
                            elem_size=DS[2],
                            transpose=False,
                            single_packet=bool(spk),
                            queue_num=nextq(),
                        )
                        for k in range(0, N3, gcap):
                            nk = min(gcap, N3 - k)
                            nc.gpsimd.dma_gather(
                                out_ap=st3[:, k // P : (k + nk) // P, :],
                                in_ap=e3_d[:],
                                idxs_ap=gidx[
                                    :, (G2 + k) // 16 : (G2 + k + nk) // 16
                                ],
                                num_idxs=nk,
                                num_idxs_reg=nk,
                                elem_size=DS[3],
                                transpose=False,
                                single_packet=bool(spk),
                                queue_num=nextq(),
                            )
                        nc.scalar.dma_start_transpose(
                            out=l2[:, :, :], in_=st2[:, :, :]
                        )
                        tsp = -(-nb3 // ts)
                        for i in range(ts):
                            b0 = i * tsp
                            b1 = min(nb3, b0 + tsp)
                            if b0 >= b1:
                                break
                            teng = nc.sync if i % 2 == 0 else nc.scalar
                            teng.dma_start_transpose(
                                out=gt3[:, b0:b1, :], in_=st3[:, b0:b1, :]
                            )
                    else:
                        nc.gpsimd.dma_gather(
                            out_ap=l2[:, :, :],
                            in_ap=e2s[:],
                            idxs_ap=gidx[:, 0 : G2 // 16],
                            num_idxs=G2,
                            num_idxs_reg=G2,
                            elem_size=2 * P,
                            transpose=True,
                            single_packet=bool(spk),
                            queue_num=nextq(),
                            sbuf_tokens_per_rank=P,
                            sbuf_free_dim_per_rank=4 * P,  # 512B rank stripe
                        )
                        for k in range(0, N3, gcap):
                            nk = min(gcap, N3 - k)
                            nc.gpsimd.dma_gather(
                                out_ap=gt3[:, :, k : k + nk],
                                in_ap=e3s[:],
                                idxs_ap=gidx[
                                    :, (G2 + k) // 16 : (G2 + k + nk) // 16
                                ],
                                num_idxs=nk,
                                num_idxs_reg=nk,
                                elem_size=P,
                                transpose=True,
                                single_packet=bool(spk),
                                queue_num=nextq(),
                                sbuf_tokens_per_rank=P,
                                sbuf_free_dim_per_rank=2 * P,  # 256B rank stripe
                            )
                elif "m" in parts:
                    for g in (gt3, l2):  # token writes so reads see an alloc
                        nc.vector.tensor_copy(out=g[:, 0, :2], in_=wcat[:, :2])
                ci = 0
                oi = 0
                while oi < t_total:
                    gb = min(gbatch, t_total - oi)
                    zt = zp.tile([P, gb, D], bf16, tag="z")
                    for g in range(gb):
                        t = oi + g
                        if ph:
                            ph0 = pp.tile([P, 512], f32, tag="ps", name="ph0")
                            ph1 = pp.tile([P, 512], f32, tag="ps", name="ph1")
                            phs = [ph0, ph1]
                        else:
                            ps = pp.tile([P, D], f32, tag="ps")
                            phs = [ps[:, 0:512], ps[:, 512:1024]]
                        if "m" in parts:
                            ch = tile_chunks(t)
                            for j, (src, cix, col) in enumerate(ch):
                                if hb:
                                    blk = col // P
                                    lhsT = (l2[:, 2 * blk + cix, :] if src == 2
                                            else gt3[:, blk, :])
                                else:
                                    buf = l2 if src == 2 else gt3
                                    lhsT = buf[:, cix, col : col + P]
                                roff = (cix if src == 2 else 2) * D
                                for h in range(2):
                                    nc.tensor.matmul(
                                        out=phs[h][:, :],
                                        lhsT=lhsT,
                                        rhs=wcat[:, roff + h * 512 :][:, :512],
                                        start=(j == 0),
                                        stop=(j == len(ch) - 1),
                                    )
                        elif "c" in parts:
                            for h in range(2):
                                nc.vector.tensor_copy(out=phs[h][:, :1], in_=wcat[:, :1])
                        if "c" in parts:
                            for h in range(2 if ph else 1):
                                dst = (zt[:, g, h * 512 : (h + 1) * 512]
                                       if ph else zt[:, g, :])
                                srcp = (phs[h] if ph else ps)[:, :]
                                if tr:
                                    # bf16 = top half-words of f32: strided
                                    # 16-bit copy converts by truncation at
                                    # 2x element rate
                                    srcp = srcp.bitcast(mybir.dt.uint16)[:, 1::2]
                                    dst = dst.bitcast(mybir.dt.uint16)
                                if cb:
                                    # batch-assigned engine: store(b) waits
                                    # only one engine's copy queue; ACT (the
                                    # slower PSUM reader) gets 3 of 8 batches
                                    on_dve = (oi // gbatch) % 8 not in (1, 4, 6)
                                else:
                                    on_dve = (ci % cr[1]) < cr[0]
                                if on_dve:
                                    nc.vector.tensor_copy(out=dst, in_=srcp)
                                else:
                                    nc.scalar.copy(out=dst, in_=srcp)
                                ci += 1
                        elif "s" in parts:
                            nc.vector.tensor_copy(out=zt[:, g, :1], in_=wcat[:, :1])
                    if "s" in parts:
                        if se == 2:
                            seng = nc.scalar if (oi // gbatch) % 2 else nc.sync
                        else:
                            seng = nc.scalar if se == 1 else nc.sync
                        seng.dma_start(out=out_d[:, oi : oi + gb, :], in_=zt[:])
                    oi += gb

            if loop_n is None:
                for _ in range(repeat):
                    body()
            else:
                # unroll inside the HW loop: the For_i epilogue is a full
                # engine barrier + sem reset, so only unrolled bodies can
                # overlap (body N+1 gathers during body N's store drain)
                unroll = u if loop_n % u == 0 else 2 if loop_n % 2 == 0 else 1
                with tc.For_i(0, loop_n // unroll, 1, staggered_reset=stag) as _i:
                    for _ in range(unroll):
                        body()
    nc.compile()
    return nc


def _prep_inputs(embs, ws, plan, mode=MODE):
    wcat = np.zeros((P, 3 * D), _BF16)
    wcat[:, 0:D] = ws[2][0:P].astype(_BF16)
    wcat[:, D : 2 * D] = ws[2][P : 2 * P].astype(_BF16)
    wcat[:, 2 * D : 3 * D] = ws[3][0:P].astype(_BF16)

    # host bucket-0/1 fold: exact f32 rows written during unshard
    p0 = embs[0].astype(np.float32) @ ws[0].astype(np.float32)
    p1 = embs[1].astype(np.float32) @ ws[1].astype(np.float32)
    hp = np.concatenate([plan.hpos[0], plan.hpos[1]])
    hv = np.concatenate([p0[plan.hloc[0]], p1[plan.hloc[1]]])
    plan.hostrows = (hp, hv)

    e2b = embs[2].astype(_BF16)
    e3b = embs[3].astype(_BF16)

    def rawwin(tab, base, R):
        # rows [base, base+R-1) zero-padded; last row(s) stay zero (Z target)
        w = np.zeros((R, tab.shape[1]), _BF16)
        nreal = min(R - 1, tab.shape[0] - base)
        w[:nreal] = tab[base : base + nreal]
        return w

    def window(tab, base, R):
        # stripe-major packing for the SBUF-resident path: row r at
        # partition r%128, rank r//128 (rank stripes along the free dim)
        w = rawwin(tab, base, R)
        return np.ascontiguousarray(
            w.reshape(R // P, P, -1).transpose(1, 0, 2).reshape(P, -1)
        )

    in_maps = []
    for c in range(NCORES):
        in_maps.append(
            {
                "e3": window(e3b, int(plan.wbase[3, c]), plan.R3),
                "e2": window(e2b, int(plan.wbase[2, c]), plan.R2),
                "e3r": rawwin(e3b, int(plan.wbase[3, c]), plan.R3),
                "e2r": rawwin(e2b, int(plan.wbase[2, c]), plan.R2),
                "wcat": wcat,
                "gidx": np.ascontiguousarray(plan.gidx[c]),
            }
        )
    return in_maps


def _assemble(plan, mode, results, repeat=1):
    out = np.empty((NTOK, D), np.float32)
    for c in range(NCORES):
        r = results[c]["out"]  # [128, T, D] partition-major
        r = np.ascontiguousarray(r.transpose(1, 0, 2)).reshape(-1, D)
        pos, slt = plan.asm[c]
        out[pos] = r[slt].astype(np.float32)
    hp, hv = plan.hostrows
    out[hp] = hv
    return out.reshape(NCORES, SEQ, D)


def run(inputs, mode=MODE, trace=False):
    x = np.asarray(inputs["x"])
    embs = [np.asarray(inputs[f"emb{b}"]) for b in range(4)]
    ws = [np.asarray(inputs[f"W{b}"]) for b in range(4)]
    assert x.shape == (NCORES, SEQ), x.shape

    plan = _plan(x)
    key = (plan.ntot, plan.G2, plan.N3, plan.R2, plan.R3, mode)
    if key not in _cache:
        _cache[key] = _build(plan, mode)
    nc = _cache[key]

    in_maps = _prep_inputs(embs, ws, plan, mode)
    res = run_bass_kernel_spmd(
        nc, in_maps, core_ids=list(range(NCORES)), trace=trace
    )
    out = _assemble(plan, mode, res.results)
    return out, res


def kernel(**inputs):
    out, _ = run(inputs, mode=MODE, trace=False)
    return out


# revision 32
# speedup vs baseline: 1.0015x; 1.0015x over previous
"""Adaptive-input-embedding Bass kernel for one TRN2 chip (8 NeuronCores). v2

Token-parallel across the 8 cores: tokens are grouped by bucket, sorted by
table index, and dealt as contiguous runs, so every core processes ~4096
tokens with identical compile-time structure.

Buckets 0/1 (300+2700 rows, ~1.1% of tokens) are folded host-side: the
host precomputes P_i = emb_i @ W_i once and writes those token rows during
unshard, so the device only handles buckets 2/3 (~99% of tokens).

Device path: each core's contiguous table windows (bucket 2: ~3.6k rows
of 256, bucket 3: ~30k rows of 128, both bf16) are SBUF-RESIDENT constants
loaded once outside the timing loop. Per body, SWDGE transpose-gathers run
SBUF->SBUF directly into lhsT layout (row r lives at partition r%128, rank
r//128 of the resident window). The gather stream's binding resource is Q7
descriptor generation (~8.7ns/row measured; the cost model's 0.34ns/desc
is wrong for gather-type gen) — spreading the calls across all 4 SWDGE
queue contexts (num_swdge_queues=4) parallelizes gen 2.9x, after which
gathers (~15us) hide under the PE stream (~20us). Matmuls against resident
bf16 W chunks accumulate into PSUM f32; PSUM copies to SBUF bf16 alternate
DVE/ACT 4:7 (PSUM has a single engine read port, so copies are
PSUM-read-bound at ~1.1/1.5us per 128x1024 tile; 4 single-bank... 4
double-bank PSUM bufs keep the mm->copy->WAR cycle off the critical path).
Output rows are stored in 4-tile batches alternating the two HWDGE rings
at the HBM roofline (~357GB/s measured): tokens hitting duplicate table
rows share one slot (host scatter fans the row out), so slots ~= unique
rows (~3968 vs 4096 tokens); the bucket-2 tail and bucket-3 head share one
mixed 128-slot tile whose PSUM accumulates both matmuls, with explicit
zero rows appended to each window keeping the off-bucket lhsT columns
zero. Residual ~8us above the
store roofline is SDMA-level interference between the gather's xbar
transpose transfers and the store stream (invariant to packet splitting,
chunk size, and queue layout; an HBM-staging + HWDGE-xbar-transpose
variant (hb=1) was slower). The host scatters returned rows to token
positions (unshard).
"""

import sys

import numpy as np

try:
    import concourse  # noqa: F401
except ImportError:
    sys.path.insert(0, "/opt/trn_rl_repo")

import ml_dtypes
from concourse import bacc, mybir, tile
from concourse.bass_utils import run_bass_kernel_spmd

BUCKETS = (0, 300, 3000, 30000, 267734)
SIZES = [BUCKETS[i + 1] - BUCKETS[i] for i in range(4)]
D = 1024
DS = [1024, 512, 256, 128]  # embedding dim per bucket
NCORES = 8
P = 128
SEQ = 4096
NTOK = NCORES * SEQ
SUB = 32768  # rows addressable by one int16 gather call
GCAP = 768  # >=1024 idxs in one SWDGE gather wedges the device

MODE = "v2"

_BF16 = ml_dtypes.bfloat16

_cache: dict = {}


def _r16(v):
    return -(-int(v) // 16) * 16


def _r128(v):
    return -(-int(v) // 128) * 128


class Plan:
    pass


def _plan(x):
    """Bucketing + even dealing of buckets 2/3 across the cores.

    Tokens of each bucket are sorted by table index and dealt as contiguous
    runs, so each core's gather indices span a narrow window of the table
    (int16-addressable, SBUF-resident). Buckets 0/1 go to the host path."""
    xf = x.reshape(-1).astype(np.int64)
    assert xf.shape[0] == NTOK
    b_arr = np.asarray(BUCKETS)
    bkt = np.clip(np.searchsorted(b_arr, xf, side="right") - 1, 0, 3)
    loc = xf - b_arr[bkt]

    p = Plan()
    # host path: buckets 0/1 (tiny token counts; host writes rows directly)
    p.hpos, p.hloc = [], []
    for b in (0, 1):
        pos = np.nonzero(bkt == b)[0]
        p.hpos.append(pos)
        p.hloc.append(loc[pos])

    percore = {}
    wbase = np.zeros((4, NCORES), np.int64)
    alloc = {}
    span = {}
    for b in (2, 3):
        pos = np.nonzero(bkt == b)[0]
        pos = pos[np.argsort(loc[pos], kind="stable")]
        n = pos.size
        cnt = np.full(NCORES, n // NCORES)
        cnt[: n % NCORES] += 1
        cuts = np.concatenate([[0], np.cumsum(cnt)])

        def spans(cuts_):
            sp, mx = 0, 0
            for c in range(NCORES):
                pc = pos[cuts_[c] : cuts_[c + 1]]
                if pc.size:
                    sp = max(sp, int(loc[pc[-1]] - loc[pc[0]]) + 1)
                    mx = max(mx, pc.size)
            return sp, mx

        sp, mx = spans(cuts)
        if b == 3 and sp > SUB - 256:
            # skewed distribution: balanced cuts straddle too-wide ranges;
            # fall back to fixed-boundary cuts (unbalanced counts but
            # windows stay int16-addressable)
            edges = np.searchsorted(loc[pos], np.arange(1, NCORES) * (SUB - 256))
            cuts = np.concatenate([[0], edges, [n]])
            sp, mx = spans(cuts)
        mxu = 16
        for c in range(NCORES):
            pc = pos[cuts[c] : cuts[c + 1]]
            lc = loc[pc]
            # dedup: tokens hitting the same table row share one slot (the
            # host scatter fans the row out to all their positions) — ~6%
            # fewer gathered rows AND stored slots
            if pc.size:
                ulc, inv = np.unique(lc, return_inverse=True)
                wbase[b, c] = ulc[0]
            else:
                ulc = np.zeros(0, np.int64)
                inv = np.zeros(0, np.int64)
            percore[(b, c)] = (pc, ulc, inv)
            mxu = max(mxu, ulc.size)
        alloc[b] = _r16(mxu)
        span[b] = max(sp, 1)

    # window shapes (global, compile-time): >=1 zero row, multiple of 128
    R2 = _r128(span[2] + 1)
    R3 = _r128(span[3] + 1)
    assert R2 <= 8192 and R3 <= SUB, (R2, R3)
    Z2, Z3 = R2 - 1, R3 - 1  # guaranteed-zero rows (windows zero-padded)

    # slot layout: b2 slots [0, A2), b3 slots [A2, ntot). The b3 gather
    # column space starts at the last 128-aligned boundary <= A2 so the
    # mixed tile accumulates b2's tail and b3's head (zero-row padding on
    # both sides keeps the off-bucket columns zero).
    A2 = alloc[2]
    T3 = (A2 // P) * P
    lead = A2 - T3
    G2 = T3 + (P if lead else 0)  # b2 gather count = r128(A2)
    N3 = _r128(lead + alloc[3])  # b3 gather count
    ntot = T3 + N3
    p.A2, p.T3, p.lead, p.G2, p.N3, p.ntot = A2, T3, lead, G2, N3, ntot
    p.R2, p.R3, p.Z2, p.Z3 = R2, R3, Z2, Z3
    p.t_total = ntot // P
    p.wbase = wbase

    NI = G2 + N3
    gidx = np.zeros((NCORES, P, NI // 16), np.int16)
    p.asm = []  # per-core (token positions, their slots) for the unshard
    for c in range(NCORES):
        idxs = np.empty(NI, np.int64)
        pc2, u2, inv2 = percore[(2, c)]
        n2 = u2.size
        idxs[:G2] = Z2
        idxs[:n2] = u2 - wbase[2, c]
        pc3, u3, inv3 = percore[(3, c)]
        n3 = u3.size
        idxs[G2:] = Z3
        idxs[G2 + lead : G2 + lead + n3] = u3 - wbase[3, c]
        p.asm.append(
            (np.concatenate([pc2, pc3]), np.concatenate([inv2, A2 + inv3]))
        )
        ii = np.arange(NI)
        cols = ii // 16
        rows = ii % 16
        for g in range(8):  # replicate across the 8 groups of 16 partitions
            gidx[c, g * 16 + rows, cols] = idxs.astype(np.int16)
    p.gidx = gidx
    return p


def _build(plan, mode=MODE, repeat=1, loop_n=None, gbatch=4, zbufs=6, psbufs=4,
           parts="gmcs", u=50, stag=False, gcap=GCAP, cr=(4, 7), spk=1, nq=4,
           se=2, ph=0, tr=0, cb=0, hb=0, ts=2, gbufs=2):
    """Build + compile the SPMD Bass program.

    repeat>1 re-emits the whole body; loop_n wraps the body in a HW For_i
    loop (both used only for differential timing). parts selects body op
    groups (g=gathers, m=matmuls, c=psum copies, s=stores). cr=(a,b):
    a of every b psum copies go to DVE, the rest to ACT."""
    bf16 = mybir.dt.bfloat16
    f32 = mybir.dt.float32
    t_total = plan.t_total
    T3, G2, N3, lead = plan.T3, plan.G2, plan.N3, plan.lead
    NI = G2 + N3

    nc = bacc.Bacc(None, target_bir_lowering=False, num_swdge_queues=nq)
    if hb:
        # raw table windows stay in HBM; gathers are non-transpose (cheap
        # single-partition row writes), lhsT built by one HWDGE xbar
        # transpose per body (the efficient 261+ GB/s path)
        e3_d = nc.declare_dram_parameter("e3r", [plan.R3, DS[3]], bf16,
                                         isOutput=False)
        e2_d = nc.declare_dram_parameter("e2r", [plan.R2, DS[2]], bf16,
                                         isOutput=False)
    else:
        e3_d = nc.declare_dram_parameter("e3", [P, plan.R3], bf16, isOutput=False)
        e2_d = nc.declare_dram_parameter("e2", [P, plan.R2 * 2], bf16,
                                         isOutput=False)
    wcat_d = nc.declare_dram_parameter("wcat", [P, 3 * D], bf16, isOutput=False)
    gidx_d = nc.declare_dram_parameter("gidx", [P, NI // 16], mybir.dt.int16,
                                       isOutput=False)
    # partition-major: slot s lives at out[s % 128, s // 128, :] so each
    # partition's store stream is contiguous (few, large descriptors)
    out_d = nc.declare_dram_parameter("out", [P, t_total, D], bf16, isOutput=True)

    with tile.TileContext(nc) as tc:
        with (
            tc.tile_pool(name="const", bufs=1) as cp,
            tc.tile_pool(name="gbuf", bufs=gbufs) as gp,
            tc.tile_pool(name="zbuf", bufs=zbufs) as zp,
            tc.tile_pool(name="ps", bufs=psbufs, space="PSUM") as pp,
        ):
            gidx = cp.tile([P, NI // 16], mybir.dt.int16)
            nc.gpsimd.dma_start(out=gidx[:], in_=gidx_d[:])
            wcat = cp.tile([P, 3 * D], bf16)
            nc.sync.dma_start(out=wcat[:], in_=wcat_d[:])
            if not hb:
                e3s = cp.tile([P, plan.R3], bf16)
                nc.sync.dma_start(out=e3s[:], in_=e3_d[:])
                e2s = cp.tile([P, plan.R2 * 2], bf16)
                nc.scalar.dma_start(out=e2s[:], in_=e2_d[:])

            def tile_chunks(t):
                # (src, chunk, col): lhsT = src[:, chunk, col:col+P]
                if t < T3 // P:
                    return [(2, 0, t * P), (2, 1, t * P)]
                if lead and t == T3 // P:
                    return [(2, 0, T3), (2, 1, T3), (3, 0, 0)]
                return [(3, 0, t * P - T3)]

            nb3 = N3 // P
            nb2 = G2 // P

            def body(_iv=None):
                # double-buffered gather tiles: body N+1's gathers overlap
                # body N's matmul/copy/store consume phase
                if hb:
                    st3 = gp.tile([P, nb3, DS[3]], bf16, tag="S3")
                    st2 = gp.tile([P, nb2, DS[2]], bf16, tag="S2")
                    gt3 = gp.tile([P, nb3, P], bf16, tag="G3")  # lhsT blocks
                    l2 = gp.tile([P, 2 * nb2, P], bf16, tag="L2")  # half-rows
                else:
                    gt3 = gp.tile([P, 1, N3], bf16, tag="G3")
                    l2 = gp.tile([P, 2, G2], bf16, tag="L2")
                if "g" in parts:
                    qn = [0]

                    def nextq():
                        q = qn[0] % nq
                        qn[0] += 1
                        return q

                    if hb:
                        # HBM non-transpose row gathers into token-major
                        # staging; one HWDGE xbar transpose per target builds
                        # the lhsT blocks (out[d, b, p] = st[p, b*128+d])
                        nc.gpsimd.dma_gather(
                            out_ap=st2[:, :, :],
                            in_ap=e2_d[:],
                            idxs_ap=gidx[:, 0 : G2 // 16],
                            num_idxs=G2,
                            num_idxs_reg=G2,
                            elem_size=DS[2],
                            transpose=False,
                            single_packet=bool(spk),
                            queue_num=nextq(),
                        )
                        for k in range(0, N3, gcap):
                            nk = min(gcap, N3 - k)
                            nc.gpsimd.dma_gather(
                                out_ap=st3[:, k // P : (k + nk) // P, :],
                                in_ap=e3_d[:],
                                idxs_ap=gidx[
                                    :, (G2 + k) // 16 : (G2 + k + nk) // 16
                                ],
                                num_idxs=nk,
                                num_idxs_reg=nk,
                                elem_size=DS[3],
                                transpose=False,
                                single_packet=bool(spk),
                                queue_num=nextq(),
                            )
                        nc.scalar.dma_start_transpose(
                            out=l2[:, :, :], in_=st2[:, :, :]
                        )
                        tsp = -(-nb3 // ts)
                        for i in range(ts):
                            b0 = i * tsp
                            b1 = min(nb3, b0 + tsp)
                            if b0 >= b1:
                                break
                            teng = nc.sync if i % 2 == 0 else nc.scalar
                            teng.dma_start_transpose(
                                out=gt3[:, b0:b1, :], in_=st3[:, b0:b1, :]
                            )
                    else:
                        nc.gpsimd.dma_gather(
                            out_ap=l2[:, :, :],
                            in_ap=e2s[:],
                            idxs_ap=gidx[:, 0 : G2 // 16],
                            num_idxs=G2,
                            num_idxs_reg=G2,
                            elem_size=2 * P,
                            transpose=True,
                            single_packet=bool(spk),
                            queue_num=nextq(),
                            sbuf_tokens_per_rank=P,
                            sbuf_free_dim_per_rank=4 * P,  # 512B rank stripe
                        )
                        for k in range(0, N3, gcap):
                            nk = min(gcap, N3 - k)
                            nc.gpsimd.dma_gather(
                                out_ap=gt3[:, :, k : k + nk],
                                in_ap=e3s[:],
                                idxs_ap=gidx[
                                    :, (G2 + k) // 16 : (G2 + k + nk) // 16
                                ],
                                num_idxs=nk,
                                num_idxs_reg=nk,
                                elem_size=P,
                                transpose=True,
                                single_packet=bool(spk),
                                queue_num=nextq(),
                                sbuf_tokens_per_rank=P,
                                sbuf_free_dim_per_rank=2 * P,  # 256B rank stripe
                            )
                elif "m" in parts:
                    for g in (gt3, l2):  # token writes so reads see an alloc
                        nc.vector.tensor_copy(out=g[:, 0, :2], in_=wcat[:, :2])
                ci = 0
                oi = 0
                while oi < t_total:
                    gb = min(gbatch, t_total - oi)
                    zt = zp.tile([P, gb, D], bf16, tag="z")
                    for g in range(gb):
                        t = oi + g
                        if ph:
                            ph0 = pp.tile([P, 512], f32, tag="ps", name="ph0")
                            ph1 = pp.tile([P, 512], f32, tag="ps", name="ph1")
                            phs = [ph0, ph1]
                        else:
                            ps = pp.tile([P, D], f32, tag="ps")
                            phs = [ps[:, 0:512], ps[:, 512:1024]]
                        if "m" in parts:
                            ch = tile_chunks(t)
                            for j, (src, cix, col) in enumerate(ch):
                                if hb:
                                    blk = col // P
                                    lhsT = (l2[:, 2 * blk + cix, :] if src == 2
                                            else gt3[:, blk, :])
                                else:
                                    buf = l2 if src == 2 else gt3
                                    lhsT = buf[:, cix, col : col + P]
                                roff = (cix if src == 2 else 2) * D
                                for h in range(2):
                                    nc.tensor.matmul(
                                        out=phs[h][:, :],
                                        lhsT=lhsT,
                                        rhs=wcat[:, roff + h * 512 :][:, :512],
                                        start=(j == 0),
                                        stop=(j == len(ch) - 1),
                                    )
                        elif "c" in parts:
                            for h in range(2):
                                nc.vector.tensor_copy(out=phs[h][:, :1], in_=wcat[:, :1])
                        if "c" in parts:
                            for h in range(2 if ph else 1):
                                dst = (zt[:, g, h * 512 : (h + 1) * 512]
                                       if ph else zt[:, g, :])
                                srcp = (phs[h] if ph else ps)[:, :]
                                if tr:
                                    # bf16 = top half-words of f32: strided
                                    # 16-bit copy converts by truncation at
                                    # 2x element rate
                                    srcp = srcp.bitcast(mybir.dt.uint16)[:, 1::2]
                                    dst = dst.bitcast(mybir.dt.uint16)
                                if cb:
                                    # batch-assigned engine: store(b) waits
                                    # only one engine's copy queue; ACT (the
                                    # slower PSUM reader) gets 3 of 8 batches
                                    on_dve = (oi // gbatch) % 8 not in (1, 4, 6)
                                else:
                                    on_dve = (ci % cr[1]) < cr[0]
                                if on_dve:
                                    nc.vector.tensor_copy(out=dst, in_=srcp)
                                else:
                                    nc.scalar.copy(out=dst, in_=srcp)
                                ci += 1
                        elif "s" in parts:
                            nc.vector.tensor_copy(out=zt[:, g, :1], in_=wcat[:, :1])
                    if "s" in parts:
                        if se == 2:
                            seng = nc.scalar if (oi // gbatch) % 2 else nc.sync
                        else:
                            seng = nc.scalar if se == 1 else nc.sync
                        seng.dma_start(out=out_d[:, oi : oi + gb, :], in_=zt[:])
                    oi += gb

            if loop_n is None:
                for _ in range(repeat):
                    body()
            else:
                # unroll inside the HW loop: the For_i epilogue is a full
                # engine barrier + sem reset, so only unrolled bodies can
                # overlap (body N+1 gathers during body N's store drain)
                unroll = u if loop_n % u == 0 else 2 if loop_n % 2 == 0 else 1
                with tc.For_i(0, loop_n // unroll, 1, staggered_reset=stag) as _i:
                    for _ in range(unroll):
                        body()
    nc.compile()
    return nc


def _prep_inputs(embs, ws, plan, mode=MODE):
    wcat = np.zeros((P, 3 * D), _BF16)
    wcat[:, 0:D] = ws[2][0:P].astype(_BF16)
    wcat[:, D : 2 * D] = ws[2][P : 2 * P].astype(_BF16)
    wcat[:, 2 * D : 3 * D] = ws[3][0:P].astype(_BF16)

    # host bucket-0/1 fold: exact f32 rows written during unshard
    p0 = embs[0].astype(np.float32) @ ws[0].astype(np.float32)
    p1 = embs[1].astype(np.float32) @ ws[1].astype(np.float32)
    hp = np.concatenate([plan.hpos[0], plan.hpos[1]])
    hv = np.concatenate([p0[plan.hloc[0]], p1[plan.hloc[1]]])
    plan.hostrows = (hp, hv)

    e2b = embs[2].astype(_BF16)
    e3b = embs[3].astype(_BF16)

    def rawwin(tab, base, R):
        # rows [base, base+R-1) zero-padded; last row(s) stay zero (Z target)
        w = np.zeros((R, tab.shape[1]), _BF16)
        nreal = min(R - 1, tab.shape[0] - base)
        w[:nreal] = tab[base : base + nreal]
        return w

    def window(tab, base, R):
        # stripe-major packing for the SBUF-resident path: row r at
        # partition r%128, rank r//128 (rank stripes along the free dim)
        w = rawwin(tab, base, R)
        return np.ascontiguousarray(
            w.reshape(R // P, P, -1).transpose(1, 0, 2).reshape(P, -1)
        )

    in_maps = []
    for c in range(NCORES):
        in_maps.append(
            {
                "e3": window(e3b, int(plan.wbase[3, c]), plan.R3),
                "e2": window(e2b, int(plan.wbase[2, c]), plan.R2),
                "e3r": rawwin(e3b, int(plan.wbase[3, c]), plan.R3),
                "e2r": rawwin(e2b, int(plan.wbase[2, c]), plan.R2),
                "wcat": wcat,
                "gidx": np.ascontiguousarray(plan.gidx[c]),
            }
        )
    return in_maps


def _assemble(plan, mode, results, repeat=1):
    out = np.empty((NTOK, D), np.float32)
    for c in range(NCORES):
        r = results[c]["out"]  # [128, T, D] partition-major
        r = np.ascontiguousarray(r.transpose(1, 0, 2)).reshape(-1, D)
        pos, slt = plan.asm[c]
        out[pos] = r[slt].astype(np.float32)
    hp, hv = plan.hostrows
    out[hp] = hv
    return out.reshape(NCORES, SEQ, D)


def run(inputs, mode=MODE, trace=False):
    x = np.asarray(inputs["x"])
    embs = [np.asarray(inputs[f"emb{b}"]) for b in range(4)]
    ws = [np.asarray(inputs[f"W{b}"]) for b in range(4)]
    assert x.shape == (NCORES, SEQ), x.shape

    plan = _plan(x)
    key = (plan.ntot, plan.G2, plan.N3, plan.R2, plan.R3, mode)
    if key not in _cache:
        _cache[key] = _build(plan, mode)
    nc = _cache[key]

    in_maps = _prep_inputs(embs, ws, plan, mode)
    res = run_bass_kernel_spmd(
        nc, in_maps, core_ids=list(range(NCORES)), trace=trace
    )
    out = _assemble(plan, mode, res.results)
    return out, res


def kernel(**inputs):
    out, _ = run(inputs, mode=MODE, trace=False)
    return out
